# revision 37
# baseline (speedup 1.0000x reference)
"""Trainium2 Bass kernel for the sparse-MoE block (top-2 of 8 experts).

Strategy: the router (a tiny [T,H]x[H,E] matmul + top-2) and the token
dispatch run on the host; the expert FFNs -- 99.97% of the FLOPs -- run on
8 NeuronCores. Sharding is F-parallel: each core holds a 512-wide slice of
the FFN intermediate dimension for ALL 8 experts, processes every expert's
gathered token group against its slice, and returns a partial down-proj
output. The host sums the 8 partials and scatter-adds into token order
with the routing weights. This is load-balanced regardless of routing.

Matmuls run in bfloat16 (inputs cast on host) with fp32 PSUM
accumulation: same 1-cycle/row PE rate as float32r but half the HBM
traffic and SBUF footprint, plus 4x faster weight loads (FWL). The
partial down-proj outputs stay fp32.
"""

import ml_dtypes
import numpy as np

import concourse.bass as bass
import concourse.tile as tile
from concourse import mybir
from concourse.bass_utils import run_bass_kernel_spmd

B, S, H, F, E = 2, 2048, 1024, 4096, 8
TOP_K = 2
NCORES = 8
FS = F // NCORES  # 512
F32R = mybir.dt.float32r
F32 = mybir.dt.float32
BF16 = mybir.dt.bfloat16
NPBF16 = ml_dtypes.bfloat16
SILU = mybir.ActivationFunctionType.Silu
MULT = mybir.AluOpType.mult


def _split_multi_waits(nc, max_waits=1):
    """This toolchain's walrus codegen supports one sync-wait per
    instruction; Tile attaches as many as needed. Hoist extras onto
    standalone NoOps just before the instruction on the same engine
    (engine streams execute in order, so semantics are preserved)."""
    total = 0
    for f in nc.m.functions:
        for bb in f.blocks:
            new_insts = []
            changed = False
            for inst in bb.instructions:
                si = inst.sync_info
                waits = list(si.on_wait) if si and si.on_wait else []
                if len(waits) > max_waits:
                    for w in waits[:-max_waits]:
                        nop = mybir.InstNoOp(
                            name=nc.get_next_instruction_name(), ins=[], outs=[]
                        )
                        nop.engine = inst.engine
                        nop.sync_info = mybir.SyncInfo(on_wait=[w], on_update=[])
                        new_insts.append(nop)
                        total += 1
                    inst.sync_info = mybir.SyncInfo(
                        on_wait=waits[-max_waits:],
                        on_update=list(si.on_update) if si.on_update else [],
                    )
                    changed = True
                new_insts.append(inst)
            if changed:
                bb.instructions = new_insts
    return total


def _expert_chunk_widths(cnt):
    # Split a token count into chunk widths <=512, keeping every chunk
    # >=256 where possible (fp32r matmuls run at full rate only when the
    # moving free dim is >=256). No padding: widths sum to cnt exactly.
    if cnt == 0:
        return []
    if cnt <= 512:
        return [cnt]
    n512, tail = divmod(cnt, 512)
    if tail == 0:
        return [512] * n512
    if tail >= 256:
        return [512] * n512 + [tail]
    # borrow from the last full chunk: 512 + tail -> (256 + tail) + 256
    return [512] * (n512 - 1) + [256 + tail, 256]


def _make_chunks(pads):
    """Chunk list in PROCESSING order. xt columns stay packed in expert order
    0..E-1; processing order is permuted so the globally-last chunk is the
    smallest one (shorter end-of-kernel drain), and the first processed
    chunk is split to <=256 tokens so the PE can start after less DMA."""
    per_e = []
    base = 0
    for e, pad in enumerate(pads):
        off = 0
        cl = []
        for w in _expert_chunk_widths(pad):
            cl.append((e, base + off, w))
            off += w
        per_e.append(cl)
        base += pad
    nonempty = [e for e in range(len(pads)) if per_e[e]]
    last = min(nonempty, key=lambda e: per_e[e][-1][2])
    order = [e for e in nonempty if e != last] + [last]
    # split the first processed chunk: 256 + rest, so the PE can start after
    # less DMA (xt0 + first gate-weight half)
    e0 = order[0]
    fe, fc0, fw = per_e[e0][0]
    if fw > 384:
        per_e[e0] = [(fe, fc0, 256), (fe, fc0 + 256, fw - 256)] + per_e[e0][1:]
    chunks = [c for e in order for c in per_e[e]]
    return chunks, base


def _build_program(pads, loop_iters=1, bufs=None, xt_eng='sync', y_eng='sync',
                   warmup_mms=52):
    bufs = {**{'w': 2, 'x': 3, 'a': 3, 'g': 3, 'y': 3, 'pg': 1, 'pu': 4, 'py': 3}, **(bufs or {})}
    chunks, CT = _make_chunks(pads)
    nc = bass.Bass("TRN2", target_bir_lowering=False, debug=False, num_devices=NCORES)
    xt = nc.declare_dram_parameter("xt", [H, CT], BF16, isOutput=False)
    wg = nc.declare_dram_parameter("wg", [E, H, FS], BF16, isOutput=False)
    wu = nc.declare_dram_parameter("wu", [E, H, FS], BF16, isOutput=False)
    wd = nc.declare_dram_parameter("wd", [E, FS, H], BF16, isOutput=False)
    yp = nc.declare_dram_parameter("yp", [CT, H], BF16, isOutput=True)

    xt3 = xt[:].rearrange("(ko p) c -> p ko c", p=128)  # [128, 8, CT]

    with tile.TileContext(nc) as tc:
        with (
            tc.tile_pool(name="wpool", bufs=bufs["w"]) as wpool,
            tc.tile_pool(name="xpool", bufs=bufs["x"]) as xpool,
            tc.tile_pool(name="apool", bufs=bufs["a"]) as apool,
            tc.tile_pool(name="gpool", bufs=bufs["g"]) as gpool,
            tc.tile_pool(name="ypool", bufs=bufs["y"]) as ypool,
            tc.tile_pool(name="pga", bufs=bufs["pg"], space="PSUM") as pg_pool,
            tc.tile_pool(name="pua", bufs=bufs["pu"], space="PSUM") as pu_pool,
            tc.tile_pool(name="pyb", bufs=bufs["py"], space="PSUM") as py_pool,
        ):
            import contextlib

            loop_ctx = (
                tc.For_i(0, loop_iters) if loop_iters > 1 else contextlib.nullcontext()
            )

            def load_weights(e, split=False):
                wgt = wpool.tile([128, H // 128, FS], BF16, tag="wg")
                wut = wpool.tile([128, H // 128, FS], BF16, tag="wu")
                wdt = wpool.tile([128, FS // 128, H], BF16, tag="wd")
                wg3 = wg[e].rearrange("(ko p) f -> p ko f", p=128)
                wu3 = wu[e].rearrange("(ko p) f -> p ko f", p=128)
                wd3 = wd[e].rearrange("(ko p) h -> p ko h", p=128)
                deferred = None
                if split:
                    # Startup: issue in consumption order on ONE queue so the
                    # first gate group (cols 0:256) can start after 1/4 of the
                    # gate weights land; up weights interleave behind. The
                    # down weights aren't needed until the next chunk -- defer
                    # them behind that chunk's x load.
                    hh = FS // 2
                    nc.sync.dma_start(wgt[:, :, :hh], wg3[:, :, :hh])
                    nc.sync.dma_start(wut[:, :, :hh], wu3[:, :, :hh])
                    nc.sync.dma_start(wgt[:, :, hh:], wg3[:, :, hh:])
                    nc.sync.dma_start(wut[:, :, hh:], wu3[:, :, hh:])
                    deferred = lambda: nc.sync.dma_start(wdt[:], wd3)
                else:
                    nc.sync.dma_start(wgt[:], wg3)
                    nc.sync.dma_start(wut[:], wu3)
                    nc.sync.dma_start(wdt[:], wd3)
                return wgt, wut, wdt, deferred

            def stage_b(act, w, c0, wdt, final=False):
                ncs = -(-w // 128)
                for cs in range(ncs):
                    m = min(128, w - cs * 128)
                    yt = ypool.tile([128, H], BF16, tag="y")
                    # on the very last token group, store each 512-col half as
                    # soon as its PSUM->SBUF copy lands (shorter end drain)
                    split_store = final and cs == ncs - 1
                    for ht in range(2):
                        py = py_pool.tile([128, 512], F32, tag="py")
                        for kf in range(FS // 128):
                            nc.tensor.matmul(
                                py[:m],
                                act[:, kf, cs * 128 : cs * 128 + m],
                                wdt[:, kf, ht * 512 : (ht + 1) * 512],
                                start=(kf == 0),
                                stop=(kf == FS // 128 - 1),
                            )
                        nc.vector.tensor_copy(yt[:m, ht * 512 : (ht + 1) * 512], py[:m])
                        if split_store:
                            getattr(nc, y_eng).dma_start(
                                yp[
                                    c0 + cs * 128 : c0 + cs * 128 + m,
                                    ht * 512 : (ht + 1) * 512,
                                ],
                                yt[:m, ht * 512 : (ht + 1) * 512],
                            )
                    if not split_store:
                        getattr(nc, y_eng).dma_start(
                            yp[c0 + cs * 128 : c0 + cs * 128 + m, :], yt[:m]
                        )

            if warmup_mms:
                # Dummy matmuls on zeroed SBUF burn the PE's ~3us p-state
                # ramp window during the startup DMA wait, so the first real
                # matmuls run at full clock. Sized to end just as the first
                # real inputs land.
                wz = gpool.tile([128, 128], BF16, tag="warm")
                nc.vector.memset(wz[:], 0)
                pwz = pg_pool.tile([128, 512], F32, tag="pg")
                for _ in range(warmup_mms):
                    nc.tensor.matmul(pwz[:, :128], wz[:], wz[:], start=True, stop=True)

            with loop_ctx:
                cur_e = -1
                wgt = wut = wdt = None
                prev = None
                deferred_wd = None
                for ci, (e, c0, w) in enumerate(chunks):
                    xtile = xpool.tile([128, H // 128, 512], BF16, tag="xt")
                    # startup chunks' x share the weight queue in consumption
                    # order (no prefetch stealing bandwidth from the
                    # critical-path startup loads); later chunks use xt_eng.
                    xq = nc.sync if ci <= 1 else getattr(nc, xt_eng)
                    xq.dma_start(xtile[:, :, :w], xt3[:, :, c0 : c0 + w])
                    if deferred_wd is not None:
                        deferred_wd()
                        deferred_wd = None
                    if e != cur_e:
                        wgt, wut, wdt, deferred_wd = load_weights(
                            e, split=(cur_e == -1)
                        )
                        cur_e = e
                    act = apool.tile([128, FS // 128, 512], BF16, tag="act")
                    for ft in range(FS // 128):
                        pg = pg_pool.tile([128, 512], F32, tag="pg")
                        pu = pu_pool.tile([128, 512], F32, tag="pu")
                        for k in range(H // 128):
                            nc.tensor.matmul(
                                pg[:, :w],
                                wgt[:, k, ft * 128 : (ft + 1) * 128],
                                xtile[:, k, :w],
                                start=(k == 0),
                                stop=(k == H // 128 - 1),
                            )
                        for k in range(H // 128):
                            nc.tensor.matmul(
                                pu[:, :w],
                                wut[:, k, ft * 128 : (ft + 1) * 128],
                                xtile[:, k, :w],
                                start=(k == 0),
                                stop=(k == H // 128 - 1),
                            )
                        gs = gpool.tile([128, 512], F32, tag="g")
                        nc.scalar.activation(gs[:, :w], pg[:, :w], SILU)
                        nc.vector.tensor_tensor(act[:, ft, :w], gs[:, :w], pu[:, :w], MULT)
                    if prev is not None:
                        stage_b(*prev)
                    prev = (act, w, c0, wdt)
                stage_b(*prev, final=True)

    _split_multi_waits(nc)
    return nc, CT


_program_cache = {}
LAST_RESULTS = None  # stashed BassKernelResults for external profiling harnesses


def _get_program(pads, loop_iters=1):
    key = (tuple(pads), loop_iters)
    if key not in _program_cache:
        _program_cache[key] = _build_program(pads, loop_iters=loop_iters)
    return _program_cache[key]


def _route(x, w_gate):
    """Host router: softmax(fp32) then top-2, matching jax.lax.top_k
    tie-breaking (lowest index first)."""
    logits = x @ w_gate  # [T, E] fp32
    m = logits.max(axis=-1, keepdims=True)
    p = np.exp(logits - m, dtype=np.float32)
    p /= p.sum(axis=-1, keepdims=True)
    order = np.argsort(-p, axis=-1, kind="stable")
    sel = order[:, :TOP_K]
    rw = np.take_along_axis(p, sel, axis=-1).astype(np.float32)
    return sel, rw


def _prepare(hidden_states, w_gate, w_gate_proj, w_up_proj, w_down_proj):
    """Host-side routing + dispatch: returns (ecnts, in_maps, unshard_ctx)."""
    x = np.asarray(hidden_states, dtype=np.float32).reshape(-1, H)
    w_gate = np.asarray(w_gate, dtype=np.float32)
    WG = np.asarray(w_gate_proj, dtype=np.float32)
    WU = np.asarray(w_up_proj, dtype=np.float32)
    WD = np.asarray(w_down_proj, dtype=np.float32)
    T = x.shape[0]

    sel, rw = _route(x, w_gate)

    idx, wtok, cnts = [], [], []
    for e in range(E):
        mask0 = sel[:, 0] == e
        mask1 = sel[:, 1] == e
        ie = np.nonzero(mask0 | mask1)[0]
        idx.append(ie)
        wtok.append(np.where(mask0[ie], rw[ie, 0], rw[ie, 1]).astype(np.float32))
        cnts.append(len(ie))

    # keep each expert's token count even (the pad column is zeros) so every
    # chunk width stays even regardless of routing.
    ecnts = [c + (c & 1) for c in cnts]
    base = np.concatenate([[0], np.cumsum(ecnts)])
    CT = int(base[-1])
    xt = np.zeros((H, CT), dtype=np.float32)
    for e in range(E):
        if cnts[e]:
            xt[:, base[e] : base[e] + cnts[e]] = x[idx[e]].T
    xt = xt.astype(NPBF16)
    WG = WG.astype(NPBF16)
    WU = WU.astype(NPBF16)
    WD = WD.astype(NPBF16)

    in_maps = []
    for c in range(NCORES):
        in_maps.append(
            {
                "xt": xt,
                "wg": np.ascontiguousarray(WG[:, :, c * FS : (c + 1) * FS]),
                "wu": np.ascontiguousarray(WU[:, :, c * FS : (c + 1) * FS]),
                "wd": np.ascontiguousarray(WD[:, c * FS : (c + 1) * FS, :]),
            }
        )
    return ecnts, in_maps, (T, idx, wtok, cnts, base)


def _finish(results, unshard_ctx):
    T, idx, wtok, cnts, base = unshard_ctx
    ysum = results[0]["yp"].astype(np.float32)
    for i in range(1, NCORES):
        ysum += results[i]["yp"].astype(np.float32)

    out = np.zeros((T, H), dtype=np.float32)
    for e in range(E):
        if cnts[e]:
            out[idx[e]] += ysum[base[e] : base[e] + cnts[e]] * wtok[e][:, None]
    return out.reshape(B, S, H).astype(np.float32)


def kernel(hidden_states, w_gate, w_gate_proj, w_up_proj, w_down_proj, loop_iters=1):
    ecnts, in_maps, ctx = _prepare(
        hidden_states, w_gate, w_gate_proj, w_up_proj, w_down_proj
    )
    nc, CT = _get_program(ecnts, loop_iters=loop_iters)
    res = run_bass_kernel_spmd(nc, in_maps, list(range(NCORES)))
    global LAST_RESULTS
    LAST_RESULTS = res
    return _finish(res.results, ctx)

